# revision 45
# baseline (speedup 1.0000x reference)
"""Character-LSTM Trainium2 kernel (8 NeuronCores, SPMD data-parallel).

Strategy (final config: _build_v2, QT=512, host-precomputed one-hots via
DMA, c in bf16, x-residual on i/g gates only ~ 281-293us, rel err 0.0170
vs 2e-2 tolerance; baseline was 414-436us)
--------
All B*S = 16384 words run one batched LSTM recurrence. Work is split across 8
cores by dealing words (sorted by descending length) round-robin so every core
sees an identical per-step active-column count A[t]; within a core, words live
in SBUF as columns of transposed state tiles. At step t only the first A[t]
columns are touched, so a word's last update lands exactly at its final
character and the surviving h columns are the output.

Gates are computed on the PE in fp8 (e4m3) DoubleRow mode, which contracts
K=256 per matmul (2 plane-major K=128 chunks via 3D APs [128, 2, n]):
  - x-side: per-vocab gate table P = W_ih @ emb[v] + b (bias folded into the
    table; the one-hot over 256 vocab ids fires exactly once per column)
    stored as fp8 P8 plus an fp8 residual R8 so x keeps ~bf16 accuracy. The
    residual matmul is emitted only for the i and g gate pairs (xr_pairs):
    f and o tolerate raw-fp8 x (numpy sweep: all-4 0.0121, i/g-only 0.0175,
    none 0.0324), saving 4 of 24 PE matmuls per tile.
  - h-side: W_hh and h quantized to fp8 (adds most of the error, ~1e-2).
Each gate 128-chunk accumulates 3 DoubleRow matmuls (P8, R8, W_hh) in fp32
PSUM pair tiles ([128, 2*512], one per gate-chunk pair, all 4 pairs open so
the PE can run ahead across the step boundary: x matmuls of step t+1 only
need the one-hot, h matmuls only need the first h8 columns of step t).
Activations run on the scalar engine over m-chunk PAIRS (biasless since the
bias lives in the table). The cell update runs on the vector engine with
2-plane APs covering both 128-halves of H=256 per instruction; c in bf16,
gates/h bf16/fp8. One-hots are precomputed on the host for every (step,
column) as fp8 [W, 2, 128, C] and DMA-streamed per q-tile (~21us/rep on
otherwise idle DMA engines) - this keeps the one-hot off the DVE queue and
off the PE's critical path entirely. (On-device alternatives: DVE is_equal
works but is ~12us slower end-to-end; gpsimd is_equal with fp8 out is a
~10x-slow software path costing ~550us - never use it.)
Measured on HW: PE is the critical engine (~280ns marginal per extra
DoubleRow matmul); bf16 x-path, per-m-ACT x_sep (v3), wider q-tiles, and
gpsimd cell offload all measured slower.
"""

import sys

if "/opt/trn_rl_repo" not in sys.path:
    sys.path.insert(0, "/opt/trn_rl_repo")

import contextlib

import numpy as np
import ml_dtypes

import concourse.bass as bass
import concourse.tile as tile
from concourse import bacc, mybir
from concourse.bass import ts
from concourse.bass_utils import run_bass_kernel_spmd

BF16 = ml_dtypes.bfloat16
NCORES = 8
B, S, W, E, H, V = 64, 256, 24, 128, 256, 256
QN = 1024  # output columns per q-tile (PSUM pair tile = [128, 2*QN] fp32)
MM = 512  # matmul out free-dim per instruction (one PSUM bank)

_PROGRAM_CACHE: dict = {}


def _plan(lens: np.ndarray, round_to: int = 1):
    """Column counts per step, uniform across cores."""
    wL = np.bincount(lens, minlength=W + 1)
    colsL = np.zeros(W + 1, np.int64)
    cum = 0
    for L in range(W, 0, -1):
        need = -(-int(wL[L]) // NCORES)
        newcum = -(-(cum + need) // round_to) * round_to
        colsL[L] = newcum - cum
        cum = newcum
    C = max(cum, 16)
    A = [int(colsL[t + 1 :].sum()) for t in range(W)]
    return colsL, C, A


def _assign(lens, chars, colsL, C):
    """Deal words into (core, column) slots, longest first."""
    order = np.argsort(-lens, kind="stable")
    wL = np.bincount(lens, minlength=W + 1)
    colmap = np.full((NCORES, C), -1, np.int64)
    col_chars = np.zeros((NCORES, C, W), np.int64)
    pos = 0
    s = 0
    for L in range(W, 0, -1):
        cnt = int(wL[L])
        if cnt:
            ids = order[pos : pos + cnt]
            pos += cnt
            k = np.arange(cnt) % NCORES
            j = s + np.arange(cnt) // NCORES
            colmap[k, j] = ids
            col_chars[k, j] = chars[ids]
        s += int(colsL[L])
    return colmap, col_chars


def _ap3(a: bass.AP, plane_stride: int, n: int) -> bass.AP:
    """Extend a [128, n] AP to [128, 2, n] with the given free plane stride."""
    ap = list(a.ap)
    assert len(ap) == 2 and ap[-1][0] == 1, ap
    return bass.AP(
        tensor=a.tensor, offset=a.offset, ap=[ap[0], [plane_stride, 2], [1, n]]
    )


def _build_program(C: int, A: list[int], reps: int = 1, pair_order=(0, 4, 2, 6),
                   mm_n: int = MM, hfin_sep: bool = True, diag: str = "",
                   oh_eng: str = "gpsimd"):
    key = (C, tuple(A), reps, tuple(pair_order), mm_n, hfin_sep, diag, oh_eng)
    if key in _PROGRAM_CACHE:
        return _PROGRAM_CACHE[key]

    dt = mybir.dt
    AF = mybir.ActivationFunctionType
    EQ = mybir.AluOpType.is_equal
    DR = mybir.MatmulPerfMode.DoubleRow
    nc = bacc.Bacc("TRN2", target_bir_lowering=False, debug=False, num_devices=NCORES)

    chf_d = nc.dram_tensor("chf", [W, C], dt.bfloat16, kind="ExternalInput")
    xp_d = nc.dram_tensor("xp", [8, 128, 256], dt.float8e4, kind="ExternalInput")
    xr_d = nc.dram_tensor("xr", [8, 128, 256], dt.float8e4, kind="ExternalInput")
    wh_d = nc.dram_tensor("wh", [8, 128, 256], dt.float8e4, kind="ExternalInput")
    iota_d = nc.dram_tensor("iota", [128, 2], dt.float32, kind="ExternalInput")
    out_d = nc.dram_tensor("out", [128, 2 * C], dt.bfloat16, kind="ExternalOutput")

    FUNCS = {0: AF.Sigmoid, 2: AF.Sigmoid, 4: AF.Tanh, 6: AF.Sigmoid}

    with tile.TileContext(nc) as tc:
        with (
            tc.tile_pool(name="const", bufs=1) as constp,
            tc.tile_pool(name="state", bufs=1) as statep,
            tc.tile_pool(name="chp", bufs=2) as chp,
            tc.tile_pool(name="oh", bufs=3) as ohp,
            tc.tile_pool(name="gates", bufs=2) as gatesp,
            tc.tile_pool(name="work", bufs=2) as workp,
            tc.tile_pool(name="psum", bufs=2, space="PSUM") as psump,
        ):
            xp_sb = [constp.tile([128, 256], dt.float8e4, tag=f"xp{m}", name=f"xp{m}") for m in range(8)]
            xr_sb = [constp.tile([128, 256], dt.float8e4, tag=f"xr{m}", name=f"xr{m}") for m in range(8)]
            wh_sb = [constp.tile([128, 256], dt.float8e4, tag=f"wh{m}", name=f"wh{m}") for m in range(8)]
            iota_sb = constp.tile([128, 2], dt.float32, tag="iota")
            for m in range(8):
                nc.sync.dma_start(out=xp_sb[m], in_=xp_d[m])
                nc.sync.dma_start(out=xr_sb[m], in_=xr_d[m])
                nc.sync.dma_start(out=wh_sb[m], in_=wh_d[m])
            nc.sync.dma_start(out=iota_sb, in_=iota_d[:])

            cst = statep.tile([128, 2 * C], dt.float32, tag="c", name="c")
            h8 = statep.tile([128, 2 * C], dt.float8e4, tag="h8", name="h8")
            hf = statep.tile([128, 2 * C], dt.bfloat16, tag="hf", name="hf")
            if diag:
                nc.vector.memset(h8[:], 0.0)
                nc.vector.memset(hf[:], 0.0)

            def w3(tile_, off, w, stride):
                return _ap3(tile_[:, off : off + w], stride, w)

            loop_cm = tc.For_i(0, reps, 1) if reps > 1 else contextlib.nullcontext()
            with loop_cm:
                for t in range(W):
                    At = A[t]
                    if At == 0:
                        break
                    At_next = A[t + 1] if t + 1 < W else 0
                    first = t == 0

                    chrep = chp.tile([128, C], dt.bfloat16, tag="chrep")
                    src = chf_d[t, :At]
                    nc.sync.dma_start(
                        out=chrep[:, :At],
                        in_=bass.AP(
                            tensor=src.tensor, offset=src.offset,
                            ap=[[0, 128]] + list(src.ap),
                        ),
                    )

                    for qlo in range(0, At, QN):
                        n = min(QN, At - qlo)
                        oh8 = ohp.tile([128, 2 * QN], dt.float8e4, tag="oh")
                        oh_e = nc.gpsimd if oh_eng == "gpsimd" else nc.vector
                        for v in range(2):
                            oh_e.tensor_scalar(
                                oh8[:, QN * v : QN * v + n],
                                chrep[:, qlo : qlo + n],
                                iota_sb[:, v : v + 1],
                                None,
                                op0=EQ,
                            )

                        pairs = [p for p in pair_order if not (first and p == 2)]
                        gts = {}
                        for ma in pairs:
                            ps = psump.tile([128, 2 * QN], dt.float32, tag="ps")
                            # stationary-major order: both mi sub-chunks per
                            # weight tile so the PE can reuse loaded weights
                            stats = [(xp_sb, True, False), (xr_sb, False, first)]
                            if not first:
                                stats.append((wh_sb, False, True))
                            if diag == "mm_min":
                                stats = stats[:1]
                                stats[0] = (stats[0][0], True, True)
                            for sb, st, sp in stats:
                                for mi, m in enumerate((ma, ma + 1)):
                                    for hh in range(0, n, mm_n):
                                        wdt = min(mm_n, n - hh)
                                        osl = ps[:, QN * mi + hh : QN * mi + hh + wdt]
                                        if sb is wh_sb:
                                            rhs = w3(h8, qlo + hh, wdt, C)
                                        else:
                                            rhs = w3(oh8, hh, wdt, QN)
                                        nc.tensor.matmul(
                                            osl,
                                            _ap3(sb[m][:, 0:128], 128, 128),
                                            rhs,
                                            start=st,
                                            stop=sp,
                                            perf_mode=DR,
                                        )
                            if diag == "no_act":
                                continue
                            gt = gatesp.tile([128, 2 * QN], dt.bfloat16, tag=f"g{ma}", name=f"g{ma}")
                            nc.scalar.activation(
                                w3(gt, 0, n, QN), w3(ps, 0, n, QN), FUNCS[ma]
                            )
                            gts[ma] = gt
                        if diag in ("no_act", "no_cell"):
                            continue

                        cap = lambda off, w: w3(cst, off, w, C)
                        if first:
                            nc.vector.tensor_mul(
                                cap(qlo, n), w3(gts[0], 0, n, QN), w3(gts[4], 0, n, QN)
                            )
                        else:
                            ig = workp.tile([128, 2 * QN], dt.bfloat16, tag="ig", name="ig")
                            nc.vector.tensor_mul(
                                w3(ig, 0, n, QN), w3(gts[0], 0, n, QN), w3(gts[4], 0, n, QN)
                            )
                            nc.vector.tensor_mul(
                                cap(qlo, n), w3(gts[2], 0, n, QN), cap(qlo, n)
                            )
                            nc.vector.tensor_add(
                                cap(qlo, n), cap(qlo, n), w3(ig, 0, n, QN)
                            )
                        th = workp.tile([128, 2 * QN], dt.bfloat16, tag="th", name="th")
                        nc.scalar.activation(w3(th, 0, n, QN), cap(qlo, n), AF.Tanh)
                        hb = min(max(At_next - qlo, 0), n)
                        if hb > 0:
                            nc.vector.tensor_mul(
                                w3(h8, qlo, hb, C),
                                w3(gts[6], 0, hb, QN),
                                w3(th, 0, hb, QN),
                            )
                        if n - hb > 0:
                            nc.vector.tensor_mul(
                                w3(hf, qlo + hb, n - hb, C),
                                w3(gts[6], hb, n - hb, QN),
                                w3(th, hb, n - hb, QN),
                            )

                nc.sync.dma_start(out=out_d[:], in_=hf[:])

    nc.compile()
    _PROGRAM_CACHE[key] = nc
    return nc


def _build_v2(C: int, A: list[int], reps: int = 1, QT: int = 512,
              oh_eng: str = "dve", psum_bufs: int = 4, dup_wh: int = 1,
              cell_eng: str = "dve", c_bf16: bool = False, x_bf16: bool = False,
              deep: bool = False, xr_pairs: tuple = (0, 2, 4, 6),
              oh_dma: bool = False, interleave: bool = False):
    """Wavefront pipeline: 512-col q-tiles, all-pairs-open PSUM, x-matmuls
    emitted before h-matmuls so the PE runs ahead across the step boundary.
    xr_pairs: gate-chunk pairs that get the fp8 x-residual matmul (0=i, 2=f,
    4=g, 6=o); dropping f/o saves 4 of 24 PE matmuls per tile at +0.005 err."""
    key = ("v2", C, tuple(A), reps, QT, oh_eng, psum_bufs, dup_wh, cell_eng,
           c_bf16, x_bf16, deep, tuple(xr_pairs), oh_dma, interleave)
    if key in _PROGRAM_CACHE:
        return _PROGRAM_CACHE[key]

    dt = mybir.dt
    AF = mybir.ActivationFunctionType
    EQ = mybir.AluOpType.is_equal
    DR = mybir.MatmulPerfMode.DoubleRow
    nc = bacc.Bacc("TRN2", target_bir_lowering=False, debug=False, num_devices=NCORES)

    if oh_dma:
        ohx_d = nc.dram_tensor("ohx", [W, 2, 128, C], dt.float8e4, kind="ExternalInput")
    else:
        chf_d = nc.dram_tensor("chf", [W, C], dt.bfloat16, kind="ExternalInput")
    if x_bf16:
        embp_d = nc.dram_tensor("embp", [2, 128, 4 * H], dt.bfloat16, kind="ExternalInput")
    else:
        xp_d = nc.dram_tensor("xp", [8, 128, 256], dt.float8e4, kind="ExternalInput")
        xr_d = nc.dram_tensor("xr", [8, 128, 256], dt.float8e4, kind="ExternalInput")
    wh_d = nc.dram_tensor("wh", [8, 128, 256], dt.float8e4, kind="ExternalInput")
    iota_d = nc.dram_tensor("iota", [128, 2], dt.float32, kind="ExternalInput")
    out_d = nc.dram_tensor("out", [128, 2 * C], dt.bfloat16, kind="ExternalOutput")

    FUNCS = {0: AF.Sigmoid, 2: AF.Sigmoid, 4: AF.Tanh, 6: AF.Sigmoid}
    oh_dt = dt.bfloat16 if x_bf16 else dt.float8e4

    with tile.TileContext(nc) as tc:
        with (
            tc.tile_pool(name="const", bufs=1) as constp,
            tc.tile_pool(name="state", bufs=1) as statep,
            tc.tile_pool(name="chp", bufs=3 if deep else 2) as chp,
            tc.tile_pool(name="oh", bufs=8 if deep else 4) as ohp,
            tc.tile_pool(name="gates", bufs=3 if deep else 2) as gatesp,
            tc.tile_pool(name="work", bufs=3 if deep else 2) as workp,
            tc.tile_pool(name="psum", bufs=psum_bufs, space="PSUM") as psump,
        ):
            if x_bf16:
                embp_sb = [constp.tile([128, 4 * H], dt.bfloat16, tag=f"ep{v}", name=f"ep{v}") for v in range(2)]
                for v in range(2):
                    nc.sync.dma_start(out=embp_sb[v], in_=embp_d[v])
            else:
                xp_sb = [constp.tile([128, 256], dt.float8e4, tag=f"xp{m}", name=f"xp{m}") for m in range(8)]
                xr_sb = [constp.tile([128, 256], dt.float8e4, tag=f"xr{m}", name=f"xr{m}") for m in range(8)]
                for m in range(8):
                    nc.sync.dma_start(out=xp_sb[m], in_=xp_d[m])
                    nc.sync.dma_start(out=xr_sb[m], in_=xr_d[m])
            wh_sb = [constp.tile([128, 256], dt.float8e4, tag=f"wh{m}", name=f"wh{m}") for m in range(8)]
            iota_sb = constp.tile([128, 2], dt.float32, tag="iota")
            for m in range(8):
                nc.sync.dma_start(out=wh_sb[m], in_=wh_d[m])
            nc.sync.dma_start(out=iota_sb, in_=iota_d[:])

            c_dt = dt.bfloat16 if c_bf16 else dt.float32
            cst = statep.tile([128, 2 * C], c_dt, tag="c", name="c")
            h8 = statep.tile([128, 2 * C], dt.float8e4, tag="h8", name="h8")
            hf = statep.tile([128, 2 * C], dt.bfloat16, tag="hf", name="hf")

            def w3(tile_, off, w, stride):
                return _ap3(tile_[:, off : off + w], stride, w)

            cell_e = nc.gpsimd if cell_eng == "mix" else nc.vector
            loop_cm = tc.For_i(0, reps, 1) if reps > 1 else contextlib.nullcontext()
            with loop_cm:
                for t in range(W):
                    At = A[t]
                    if At == 0:
                        break
                    At_next = A[t + 1] if t + 1 < W else 0
                    first = t == 0

                    if not oh_dma:
                        chrep = chp.tile([128, C], dt.bfloat16, tag="chrep")
                        src = chf_d[t, :At]
                        nc.sync.dma_start(
                            out=chrep[:, :At],
                            in_=bass.AP(
                                tensor=src.tensor, offset=src.offset,
                                ap=[[0, 128]] + list(src.ap),
                            ),
                        )

                    pairs = [0, 4, 2, 6] if not first else [0, 4, 6]
                    oh_e = nc.gpsimd if oh_eng == "gpsimd" else nc.vector
                    for qlo in range(0, At, QT):
                        n = min(QT, At - qlo)
                        oh8 = ohp.tile([128, 2 * QT], oh_dt, tag="oh")
                        if oh_dma:
                            for v in range(2):
                                nc.sync.dma_start(
                                    out=oh8[:, QT * v : QT * v + n],
                                    in_=ohx_d[t, v, :, qlo : qlo + n],
                                )
                        else:
                            for v in range(2):
                                oh_e.tensor_scalar(
                                    oh8[:, QT * v : QT * v + n],
                                    chrep[:, qlo : qlo + n],
                                    iota_sb[:, v : v + 1],
                                    None,
                                    op0=EQ,
                                )

                        pstiles = {
                            ma: psump.tile([128, 2 * QT], dt.float32, tag="ps", name=f"ps{ma}")
                            for ma in pairs
                        }
                        hhs = [(hh, min(MM, n - hh)) for hh in range(0, n, MM)]
                        if x_bf16:
                            for v in range(2):
                                for ma in pairs:
                                    for mi in range(2):
                                        for hh, wdt in hhs:
                                            nc.tensor.matmul(
                                                pstiles[ma][:, QT * mi + hh : QT * mi + hh + wdt],
                                                embp_sb[v][:, ts(ma + mi, 128)],
                                                oh8[:, QT * v + hh : QT * v + hh + wdt],
                                                start=v == 0,
                                                stop=v == 1 and first,
                                            )
                        else:
                            groups = [pairs] if interleave else [[ma] for ma in pairs]
                            # interleave: emit xp/xr/wh per pair so each
                            # pair's ACT can fire as early as possible;
                            # otherwise all x-matmuls precede any wh.
                            def emit_x(mas):
                                for ma in mas:
                                    for mi in range(2):
                                        for hh, wdt in hhs:
                                            nc.tensor.matmul(
                                                pstiles[ma][:, QT * mi + hh : QT * mi + hh + wdt],
                                                _ap3(xp_sb[ma + mi][:, 0:128], 128, 128),
                                                w3(oh8, hh, wdt, QT),
                                                start=True,
                                                stop=first and ma not in xr_pairs,
                                                perf_mode=DR,
                                            )
                                for ma in mas:
                                    if ma not in xr_pairs:
                                        continue
                                    for mi in range(2):
                                        for hh, wdt in hhs:
                                            nc.tensor.matmul(
                                                pstiles[ma][:, QT * mi + hh : QT * mi + hh + wdt],
                                                _ap3(xr_sb[ma + mi][:, 0:128], 128, 128),
                                                w3(oh8, hh, wdt, QT),
                                                start=False,
                                                stop=first,
                                                perf_mode=DR,
                                            )

                            def emit_wh(mas):
                                if first:
                                    return
                                for d in range(dup_wh):
                                    for ma in mas:
                                        for mi in range(2):
                                            for hh, wdt in hhs:
                                                nc.tensor.matmul(
                                                    pstiles[ma][:, QT * mi + hh : QT * mi + hh + wdt],
                                                    _ap3(wh_sb[ma + mi][:, 0:128], 128, 128),
                                                    w3(h8, qlo + hh, wdt, C),
                                                    start=False,
                                                    stop=d == dup_wh - 1,
                                                    perf_mode=DR,
                                                )

                            if interleave:
                                for ma in pairs:
                                    emit_x([ma])
                                    emit_wh([ma])
                            else:
                                emit_x(pairs)
                                emit_wh(pairs)
                        if not first and x_bf16:
                            for d in range(dup_wh):
                                for ma in pairs:
                                    for mi in range(2):
                                        for hh, wdt in hhs:
                                            nc.tensor.matmul(
                                                pstiles[ma][:, QT * mi + hh : QT * mi + hh + wdt],
                                                _ap3(wh_sb[ma + mi][:, 0:128], 128, 128),
                                                w3(h8, qlo + hh, wdt, C),
                                                start=False,
                                                stop=d == dup_wh - 1,
                                                perf_mode=DR,
                                            )
                        gts = {}
                        for ma in pairs:
                            gt = gatesp.tile([128, 2 * QT], dt.bfloat16, tag=f"g{ma}", name=f"g{ma}")
                            nc.scalar.activation(
                                w3(gt, 0, n, QT), w3(pstiles[ma], 0, n, QT), FUNCS[ma]
                            )
                            gts[ma] = gt

                        cap = lambda off, w: w3(cst, off, w, C)
                        if first:
                            nc.vector.tensor_mul(
                                cap(qlo, n), w3(gts[0], 0, n, QT), w3(gts[4], 0, n, QT)
                            )
                        else:
                            ig = workp.tile([128, 2 * QT], dt.bfloat16, tag="ig", name="ig")
                            ig_e = nc.gpsimd if cell_eng == "ig_pool" else nc.vector
                            ig_e.tensor_mul(
                                w3(ig, 0, n, QT), w3(gts[0], 0, n, QT), w3(gts[4], 0, n, QT)
                            )
                            c_e = nc.vector if cell_eng == "ig_pool" else cell_e
                            c_e.tensor_mul(
                                cap(qlo, n), w3(gts[2], 0, n, QT), cap(qlo, n)
                            )
                            c_e.tensor_add(
                                cap(qlo, n), cap(qlo, n), w3(ig, 0, n, QT)
                            )
                        th = workp.tile([128, 2 * QT], dt.bfloat16, tag="th", name="th")
                        nc.scalar.activation(w3(th, 0, n, QT), cap(qlo, n), AF.Tanh)
                        hb = min(max(At_next - qlo, 0), n)
                        if hb > 0:
                            nc.vector.tensor_mul(
                                w3(h8, qlo, hb, C),
                                w3(gts[6], 0, hb, QT),
                                w3(th, 0, hb, QT),
                            )
                        if n - hb > 0:
                            nc.vector.tensor_mul(
                                w3(hf, qlo + hb, n - hb, C),
                                w3(gts[6], hb, n - hb, QT),
                                w3(th, hb, n - hb, QT),
                            )

                nc.sync.dma_start(out=out_d[:], in_=hf[:])

    nc.compile()
    _PROGRAM_CACHE[key] = nc
    return nc


def _build_v3(C: int, A: list[int], reps: int = 1, QT: int = 1024,
              c_bf16: bool = True, psum_bufs: int = 3):
    """x-separated hybrid: bf16 one-hot gather x = emb.T @ oh (K=256), bf16
    x-matmul W_ih.T @ x per gate chunk (bias via per-m ACT bias), fp8
    DoubleRow h-matmul (K=256 in one MM). Exact-bf16 x path, fp8 h only."""
    key = ("v3", C, tuple(A), reps, QT, c_bf16, psum_bufs)
    if key in _PROGRAM_CACHE:
        return _PROGRAM_CACHE[key]

    dt = mybir.dt
    AF = mybir.ActivationFunctionType
    EQ = mybir.AluOpType.is_equal
    DR = mybir.MatmulPerfMode.DoubleRow
    nc = bacc.Bacc("TRN2", target_bir_lowering=False, debug=False, num_devices=NCORES)

    chf_d = nc.dram_tensor("chf", [W, C], dt.bfloat16, kind="ExternalInput")
    embc_d = nc.dram_tensor("embc", [2, 128, E], dt.bfloat16, kind="ExternalInput")
    wih_d = nc.dram_tensor("wih", [E, 4 * H], dt.bfloat16, kind="ExternalInput")
    bias_d = nc.dram_tensor("bias", [128, 8], dt.float32, kind="ExternalInput")
    wh_d = nc.dram_tensor("wh", [8, 128, 256], dt.float8e4, kind="ExternalInput")
    iota_d = nc.dram_tensor("iota", [128, 2], dt.float32, kind="ExternalInput")
    out_d = nc.dram_tensor("out", [128, 2 * C], dt.bfloat16, kind="ExternalOutput")

    FUNCS = {0: AF.Sigmoid, 2: AF.Sigmoid, 4: AF.Tanh, 6: AF.Sigmoid}

    with tile.TileContext(nc) as tc:
        with (
            tc.tile_pool(name="const", bufs=1) as constp,
            tc.tile_pool(name="state", bufs=1) as statep,
            tc.tile_pool(name="chp", bufs=2) as chp,
            tc.tile_pool(name="oh", bufs=3) as ohp,
            tc.tile_pool(name="gates", bufs=2) as gatesp,
            tc.tile_pool(name="work", bufs=2) as workp,
            tc.tile_pool(name="psum", bufs=psum_bufs, space="PSUM") as psump,
            tc.tile_pool(name="xps", bufs=2, space="PSUM") as xpsp,
        ):
            embc_sb = [constp.tile([128, E], dt.bfloat16, tag=f"ec{v}", name=f"ec{v}") for v in range(2)]
            wih_sb = constp.tile([E, 4 * H], dt.bfloat16, tag="wih")
            bias_sb = constp.tile([128, 8], dt.float32, tag="bias")
            wh_sb = [constp.tile([128, 256], dt.float8e4, tag=f"wh{m}", name=f"wh{m}") for m in range(8)]
            iota_sb = constp.tile([128, 2], dt.float32, tag="iota")
            for v in range(2):
                nc.sync.dma_start(out=embc_sb[v], in_=embc_d[v])
            nc.sync.dma_start(out=wih_sb, in_=wih_d[:])
            nc.sync.dma_start(out=bias_sb, in_=bias_d[:])
            for m in range(8):
                nc.sync.dma_start(out=wh_sb[m], in_=wh_d[m])
            nc.sync.dma_start(out=iota_sb, in_=iota_d[:])

            c_dt = dt.bfloat16 if c_bf16 else dt.float32
            cst = statep.tile([128, 2 * C], c_dt, tag="c", name="c")
            h8 = statep.tile([128, 2 * C], dt.float8e4, tag="h8", name="h8")
            hf = statep.tile([128, 2 * C], dt.bfloat16, tag="hf", name="hf")

            def w3(tile_, off, w, stride):
                return _ap3(tile_[:, off : off + w], stride, w)

            loop_cm = tc.For_i(0, reps, 1) if reps > 1 else contextlib.nullcontext()
            with loop_cm:
                for t in range(W):
                    At = A[t]
                    if At == 0:
                        break
                    At_next = A[t + 1] if t + 1 < W else 0
                    first = t == 0

                    chrep = chp.tile([128, C], dt.bfloat16, tag="chrep")
                    src = chf_d[t, :At]
                    nc.sync.dma_start(
                        out=chrep[:, :At],
                        in_=bass.AP(
                            tensor=src.tensor, offset=src.offset,
                            ap=[[0, 128]] + list(src.ap),
                        ),
                    )

                    morder = [0, 1, 4, 5, 2, 3, 6, 7] if not first else [0, 1, 4, 5, 6, 7]
                    for qlo in range(0, At, QT):
                        n = min(QT, At - qlo)
                        oh = ohp.tile([128, 2 * QT], dt.bfloat16, tag="oh")
                        for v in range(2):
                            nc.vector.tensor_scalar(
                                oh[:, QT * v : QT * v + n],
                                chrep[:, qlo : qlo + n],
                                iota_sb[:, v : v + 1],
                                None,
                                op0=EQ,
                            )
                        # x = emb.T @ onehot  (K=256 over two vocab chunks)
                        xall = workp.tile([128, QT], dt.bfloat16, tag="xall", name="xall")
                        for hh in range(0, n, MM):
                            wdt = min(MM, n - hh)
                            xps = xpsp.tile([128, MM], dt.float32, tag="xps", name="xps")
                            for v in range(2):
                                nc.tensor.matmul(
                                    xps[:, :wdt],
                                    embc_sb[v][:],
                                    oh[:, QT * v + hh : QT * v + hh + wdt],
                                    start=v == 0,
                                    stop=v == 1,
                                )
                            nc.vector.tensor_copy(
                                xall[:, hh : hh + wdt], xps[:, :wdt]
                            )

                        gts = {}
                        for m in morder:
                            ma, mi = (m // 2) * 2, m % 2
                            ps = psump.tile([128, QT], dt.float32, tag="ps", name="ps")
                            for hh in range(0, n, MM):
                                wdt = min(MM, n - hh)
                                nc.tensor.matmul(
                                    ps[:, hh : hh + wdt],
                                    wih_sb[:, ts(m, 128)],
                                    xall[:, hh : hh + wdt],
                                    start=True,
                                    stop=first,
                                )
                                if not first:
                                    nc.tensor.matmul(
                                        ps[:, hh : hh + wdt],
                                        _ap3(wh_sb[m][:, 0:128], 128, 128),
                                        w3(h8, qlo + hh, wdt, C),
                                        start=False,
                                        stop=True,
                                        perf_mode=DR,
                                    )
                            if mi == 0:
                                gts[ma] = gatesp.tile(
                                    [128, 2 * QT], dt.bfloat16, tag=f"g{ma}", name=f"g{ma}"
                                )
                            nc.scalar.activation(
                                gts[ma][:, QT * mi : QT * mi + n],
                                ps[:, :n],
                                FUNCS[ma],
                                bias=bias_sb[:, m : m + 1],
                            )

                        cap = lambda off, w: w3(cst, off, w, C)
                        if first:
                            nc.vector.tensor_mul(
                                cap(qlo, n), w3(gts[0], 0, n, QT), w3(gts[4], 0, n, QT)
                            )
                        else:
                            ig = workp.tile([128, 2 * QT], dt.bfloat16, tag="ig", name="ig")
                            nc.vector.tensor_mul(
                                w3(ig, 0, n, QT), w3(gts[0], 0, n, QT), w3(gts[4], 0, n, QT)
                            )
                            nc.vector.tensor_mul(
                                cap(qlo, n), w3(gts[2], 0, n, QT), cap(qlo, n)
                            )
                            nc.vector.tensor_add(
                                cap(qlo, n), cap(qlo, n), w3(ig, 0, n, QT)
                            )
                        th = workp.tile([128, 2 * QT], dt.bfloat16, tag="th", name="th")
                        nc.scalar.activation(w3(th, 0, n, QT), cap(qlo, n), AF.Tanh)
                        hb = min(max(At_next - qlo, 0), n)
                        if hb > 0:
                            nc.vector.tensor_mul(
                                w3(h8, qlo, hb, C),
                                w3(gts[6], 0, hb, QT),
                                w3(th, 0, hb, QT),
                            )
                        if n - hb > 0:
                            nc.vector.tensor_mul(
                                w3(hf, qlo + hb, n - hb, C),
                                w3(gts[6], hb, n - hb, QT),
                                w3(th, hb, n - hb, QT),
                            )

                nc.sync.dma_start(out=out_d[:], in_=hf[:])

    nc.compile()
    _PROGRAM_CACHE[key] = nc
    return nc


def _prepare(char_input, embedding, W_ih, W_hh, b_ih, b_hh, round_to=8):
    ci = np.asarray(char_input)
    chars = ci.reshape(-1, W).astype(np.int64)
    lens = (chars != 0).sum(-1)

    colsL, C, A = _plan(lens, round_to)
    colmap, col_chars = _assign(lens, chars, colsL, C)

    F8 = mybir.dt.np(mybir.dt.float8e4)
    emb = np.asarray(embedding, np.float32)
    bias = (np.asarray(b_ih) + np.asarray(b_hh)).astype(np.float32)
    P = emb @ np.asarray(W_ih, np.float32).T + bias  # [V, 4H]
    P8 = P.astype(F8)
    R8 = (P - P8.astype(np.float32)).astype(F8)

    def xlayout(tbl):  # [V, 4H] -> [8 m, 128 k, 2*128 (i,j)]
        a = np.asarray(tbl).reshape(2, 128, 8, 128)  # [i, k, m, j]
        return np.ascontiguousarray(a.transpose(2, 1, 0, 3).reshape(8, 128, 256))

    W8 = np.asarray(W_hh, np.float32).astype(F8)
    b_ = W8.reshape(8, 128, 2, 128)  # [m, j, i, k]
    wh = np.ascontiguousarray(b_.transpose(0, 3, 2, 1).reshape(8, 128, 256))

    iota = np.ascontiguousarray(
        (np.arange(128)[:, None] + np.array([0, 128])[None, :]).astype(np.float32)
    )
    common = {
        "xp": xlayout(P8), "xr": xlayout(R8), "wh": wh, "iota": iota,
        "embc": np.ascontiguousarray(emb.astype(BF16).reshape(2, 128, E)),
        "wih": np.ascontiguousarray(np.asarray(W_ih, np.float32).T.astype(BF16)),
        "bias": np.ascontiguousarray(bias.reshape(8, 128).T),
        "embp": np.ascontiguousarray(P.astype(BF16).reshape(2, 128, 4 * H)),
    }
    ids = np.arange(V)
    in_maps = []
    for k in range(NCORES):
        chf = np.ascontiguousarray(col_chars[k].T.astype(BF16))  # [W, C]
        ch = col_chars[k].T  # [W, C]
        ohx = np.ascontiguousarray(
            (ch[:, None, :] == ids[None, :, None]).astype(F8).reshape(W, 2, 128, C)
        )
        in_maps.append({"chf": chf, "ohx": ohx, **common})
    return colmap, in_maps, C, A


def _gather_output(results, colmap):
    C = colmap.shape[1]
    out_flat = np.zeros((B * S, H), np.float32)
    for k in range(NCORES):
        o = np.asarray(results[k]["out"]).astype(np.float32)  # [128, 2C]
        h_core = np.concatenate([o[:, :C], o[:, C:]], axis=0)  # [H, C]
        mask = colmap[k] >= 0
        out_flat[colmap[k][mask]] = h_core[:, mask].T
    return out_flat.reshape(B, S, H)


def kernel(char_input, embedding, W_ih, W_hh, b_ih, b_hh):
    colmap, in_maps, C, A = _prepare(
        char_input, embedding, W_ih, W_hh, b_ih, b_hh, round_to=8
    )
    nc = _build_v2(C, A, c_bf16=True, xr_pairs=(0, 4), oh_dma=True)
    res = run_bass_kernel_spmd(nc, in_maps, core_ids=list(range(NCORES)))
    return _gather_output(res.results, colmap)
